# revision 26
# baseline (speedup 1.0000x reference)
"""Trainium2 Bass kernel for nn_CausalFT (causal Fourier transform + residual + LayerNorm).

reference semantics (QLEN=1024, MLEN=1024, BATCH=8, D_MODEL=1024, klen=2048):
    cat  = concat([mems, dec_inp], axis=0) (+ pos_emb broadcast over batch)
    ft   = einsum('ml,lbd->mbd', ft_matrix(1024, 2048), cat)
    x    = dec_inp + ft / sqrt(2048)
    out  = LayerNorm_d(x) * gamma + beta

Sharding: data-parallel over batch — core b computes out[:, b, :] entirely
(no collectives).  The FT matrix is an input-independent constant, computed
host-side (bit-identical to the reference, via jax on CPU), pre-scaled by
1/sqrt(klen), transposed and band-packed: row-tile t of the output only needs
contraction tiles k in [t, t+8] (the matrix is banded: ft[m, j] != 0 only for
m <= j <= m+1024), which cuts matmul work and weight traffic by ~44%.

Matmuls run in float32r (1 PE cycle/row vs 4 for fp32; measured output
rel-err 7e-6 vs 6e-6 for full fp32 on this problem).
"""

import math

import numpy as np

QLEN, MLEN, BATCH, D = 1024, 1024, 8, 1024
KLEN = QLEN + MLEN
NT = QLEN // 128          # 8 output row tiles
NK = KLEN // 128          # 16 contraction tiles
BW = NK - NT + 1          # 9 band K-tiles per output row tile
LN_EPS = 1e-5

# matmul operand dtype: "f32" (exact, 4 cyc/row) or "f32r" (reduced, 1 cyc/row)
MM_DTYPE = "f32r"

_WBAND = None
_PROGS = {}


def _ft_matrix_np():
    """Replicate reference._ft_matrix bit-for-bit using jax on CPU."""
    import jax
    import jax.numpy as jnp

    cpu = jax.local_devices(backend="cpu")[0]
    with jax.default_device(cpu):
        qlen, klen = QLEN, KLEN
        ft_len = klen - qlen + 1
        m = jnp.arange(qlen, dtype=jnp.float32)
        k = jnp.arange(ft_len, dtype=jnp.float32)
        base = jnp.cos((2.0 * float(np.pi)) * jnp.outer(m, k) / float(ft_len))
        base = base / float(np.sqrt(ft_len))
        mat = jnp.pad(base, ((0, 0), (klen - ft_len, 0)))
        shift = (qlen - 1) - jnp.arange(qlen)
        cols = (jnp.arange(klen)[None, :] + shift[:, None]) % klen
        mat = jnp.take_along_axis(mat, cols, axis=1)
        rows = jnp.arange(qlen)[:, None]
        js = jnp.arange(klen)[None, :]
        mask = (js <= rows + (klen - qlen)) & (js >= rows)
        mat = jnp.where(mask, mat, jnp.float32(0.0))
        return np.asarray(jax.device_get(mat), dtype=np.float32)


def _wband():
    """Band-packed, pre-scaled, transposed FT matrix: [128, NT*BW, 128].

    wband[p, t*BW+i, m] = (W/sqrt(KLEN))[128t+m, 128(t+i)+p]  (lhsT layout).
    """
    global _WBAND
    if _WBAND is None:
        w = _ft_matrix_np() / np.float32(math.sqrt(KLEN))
        wb = np.empty((128, NT * BW, 128), dtype=np.float32)
        for t in range(NT):
            for i in range(BW):
                blk = w[128 * t : 128 * (t + 1), 128 * (t + i) : 128 * (t + i) + 128]
                wb[:, t * BW + i, :] = blk.T
        _WBAND = np.ascontiguousarray(wb)
    return _WBAND


def _install_drain_patch():
    """Work around walrus 'Too many sync wait commands' on the Tile tail drain.

    The stock TileContext._drain_and_barrier emits ONE sync-engine Drain
    carrying a sem wait for every proc lane that ticked (up to 27).  The
    walrus build in this environment accepts only a single sync-wait per
    instruction, so split the global-clock wait set across consecutive
    Drains (one wait each) — sequential execution on the same engine gives
    the same quiescence guarantee.  Also skip the tail per-sem zeroing:
    the bass preamble range-clears every kernel semaphore at program start
    on each execution, so the ~250 walrus-expanded tail EVSEMs (~8us) are
    redundant for re-execution correctness.
    """
    import re

    import bass_rust
    import concourse.tile as _tile
    from concourse.vector_clock import ScopedClock

    if getattr(_tile.TileContext, "_drain_patch_installed", False):
        return

    def _clock_ticks(vc):
        m = re.search(r"\[([0-9, ]*)\]", repr(vc))
        if not m or not m.group(1).strip():
            return []
        return [int(x) for x in m.group(1).split(",")]

    def _patched_drain_and_barrier(self, tick_clock, wait_clock):
        nc = self.nc
        ticks = _clock_ticks(tick_clock.global_clock)
        for i, t in enumerate(ticks):
            if t > 0:
                part = bass_rust.VectorClock()
                part.require_at_least(i, t)
                d = nc.sync.drain()
                wait_clock.add_sem_waits(d.ins, ScopedClock({None: part}))
        assert self.sems is not None
        popped = nc._tile_sem_poison_stack.pop()
        assert popped is self._sem_poison
        nc._state.prepend_free_semaphores(
            [s.num for s in self.sems.allocated().values()]
        )

    _tile.TileContext._drain_and_barrier = _patched_drain_and_barrier
    _tile.TileContext._drain_patch_installed = True


def _split_excess_waits(nc, cap=1):
    """Hoist excess per-instruction sem waits onto preceding same-engine nops.

    The walrus build here accepts only `cap` sync-wait commands per
    instruction.  Engines execute their instruction stream in order, so
    moving waits to immediately-preceding same-engine nops preserves the
    ordering semantics (the instruction still starts only after every wait
    is satisfied).
    """
    import concourse.mybir as mybir

    for bb in nc.main_func.blocks:
        insts = list(bb.instructions)
        if not any(
            i.sync_info and i.sync_info.on_wait and len(i.sync_info.on_wait) > cap
            for i in insts
        ):
            continue
        new = []
        for inst in insts:
            si = inst.sync_info
            waits = list(si.on_wait) if si and si.on_wait else []
            if len(waits) > cap:
                for sw in waits[:-cap]:
                    nop = nc.engines[inst.engine].nop(nofuse=True).ins
                    cur = nc.cur_bb.bb
                    assert cur.instructions and cur.instructions[-1] is nop
                    cur.instructions.pop()
                    nop.sync_info = mybir.SyncInfo(on_wait=[sw], on_update=[])
                    new.append(nop)
                inst.sync_info = mybir.SyncInfo(
                    on_wait=waits[-cap:], on_update=list(si.on_update or [])
                )
            new.append(inst)
        bb.instructions.clear()
        for i in new:
            bb.instructions.append(i)


def _build_program(add_pos: bool, trivial_affine: bool):
    _install_drain_patch()
    import concourse.bass as bass
    import concourse.mybir as mybir
    import concourse.tile as tile

    f32 = mybir.dt.float32
    nc = bass.Bass()

    dec = nc.dram_tensor("dec", [QLEN, D], f32, kind="ExternalInput")
    memsb = nc.dram_tensor("memsb", [MLEN, D], f32, kind="ExternalInput")
    wband = nc.dram_tensor(
        "wband", [128, NT * BW, 128], mybir.dt.bfloat16, kind="ExternalInput"
    )
    pos = None
    if add_pos:
        pos = nc.dram_tensor("pos", [KLEN, D], f32, kind="ExternalInput")
    gam = bet = None
    if not trivial_affine:
        gam = nc.dram_tensor("gam", [D], f32, kind="ExternalInput")
        bet = nc.dram_tensor("bet", [D], f32, kind="ExternalInput")
    out = nc.dram_tensor("out", [QLEN, D], f32, kind="ExternalOutput")

    dec_t = dec.rearrange("(k p) d -> k p d", p=128)      # [8, 128, 1024]
    mems_t = memsb.rearrange("(k p) d -> k p d", p=128)   # [8, 128, 1024]
    out_t = out.rearrange("(t p) d -> t p d", p=128)      # [8, 128, 1024]

    # FP32r: the PE runs 4x faster, but the BIR verifier requires every
    # producer feeding an fp32r matmul to emit fp32r — so type the weight and
    # rhs tiles (and the DMA source APs, a pure bitcast) as float32r.
    mm_dt = f32 if MM_DTYPE == "f32" else mybir.dt.float32r
    src_cast = (lambda ap: ap) if MM_DTYPE == "f32" else (
        lambda ap: ap.bitcast(mybir.dt.float32r)
    )

    with tile.TileContext(nc) as tc:
        with (
            tc.tile_pool(name="big", bufs=1) as big,
            tc.tile_pool(name="posp", bufs=2) as posp,
            tc.tile_pool(name="small", bufs=1) as small,
            tc.tile_pool(name="work", bufs=3) as work,
            tc.tile_pool(name="stat", bufs=4) as stat,
            tc.tile_pool(name="ps", bufs=2, space="PSUM") as ps,
        ):
            eps = small.tile([128, 1], f32)
            nc.vector.memset(eps, LN_EPS)
            gam_sb = bet_sb = None
            if not trivial_affine:
                gam_sb = small.tile([128, D], f32)
                bet_sb = small.tile([128, D], f32)
                gam_ap, bet_ap = gam[:], bet[:]
                nc.sync.dma_start(
                    out=gam_sb,
                    in_=bass.AP(tensor=gam_ap.tensor, offset=0, ap=[[0, 128]] + list(gam_ap.ap)),
                )
                nc.sync.dma_start(
                    out=bet_sb,
                    in_=bass.AP(tensor=bet_ap.tensor, offset=0, ap=[[0, 128]] + list(bet_ap.ap)),
                )

            # --- weights: band-packed lhsT, shipped as bf16 (half the HBM
            # bytes; the rounding is ~2e-5 on the output since W carries a
            # 1/sqrt(1025*2048) scale) and up-converted to f32r on GpSimd.
            # Chunk t == the 9 tiles of matmul group t, so conversion order
            # matches consumption order.  t=0 first, rest after mems/pos. ---
            wb = big.tile([128, NT * BW, 128], mm_dt)

            def wb_chunk(t):
                wst = posp.tile([128, BW, 128], mybir.dt.bfloat16, tag="wst", bufs=2)
                nc.gpsimd.dma_start(out=wst, in_=wband[:, BW * t : BW * (t + 1), :])
                nc.scalar.copy(out=wb[:, BW * t : BW * (t + 1), :], in_=wst)

            wb_chunk(0)

            # --- rhs: catp[k] = cat tile k (+ pos), decraw for the residual.
            # Interleave mems/pos loads and the pos-adds per k so low-k catp
            # tiles (which gate the first matmul groups) complete earliest.
            catp = big.tile([128, NK, D], mm_dt)
            decraw = big.tile([128, NT, D], f32)
            pos_k = pos.rearrange("(k p) d -> k p d", p=128) if add_pos else None

            def pos_add(k, in0):
                # GpSimd takes every third add (2x slower than DVE but runs
                # in parallel; the add chain paces the matmul groups)
                pc = posp.tile([128, D], f32, tag="pos", bufs=4)
                nc.scalar.dma_start(out=pc, in_=pos_k[k])
                eng = nc.gpsimd if k % 3 == 2 else nc.vector
                eng.tensor_add(out=catp[:, k, :], in0=in0, in1=pc)

            for k in range(8):
                nc.sync.dma_start(out=catp[:, k, :], in_=src_cast(mems_t[k]))
                if add_pos:
                    pos_add(k, catp[:, k, :])
            nc.sync.dma_start(out=decraw[:, 0, :], in_=dec_t[0])
            if add_pos:
                pos_add(8, decraw[:, 0, :])
            else:
                nc.gpsimd.tensor_copy(out=catp[:, 8, :], in_=decraw[:, 0, :])
            for t in (1, 2, 3, 4):
                wb_chunk(t)
            for k in range(9, 16):
                nc.sync.dma_start(out=decraw[:, k - 8, :], in_=dec_t[k - 8])
                if add_pos:
                    pos_add(k, decraw[:, k - 8, :])
                else:
                    nc.gpsimd.tensor_copy(out=catp[:, k, :], in_=decraw[:, k - 8, :])
            for t in (5, 6, 7):
                wb_chunk(t)

            # --- band matmul + fused residual/LayerNorm epilogue per row tile ---
            for t in range(NT):
                psA = ps.tile([128, 512], f32, tag="A", bufs=4)
                psB = ps.tile([128, 512], f32, tag="B", bufs=4)
                for i in range(BW):
                    nc.tensor.matmul(
                        psA, wb[:, t * BW + i, :], catp[:, t + i, 0:512],
                        start=(i == 0), stop=(i == BW - 1),
                    )
                for i in range(BW):
                    nc.tensor.matmul(
                        psB, wb[:, t * BW + i, :], catp[:, t + i, 512:1024],
                        start=(i == 0), stop=(i == BW - 1),
                    )

                x = work.tile([128, D], f32, tag="x", bufs=4)
                nc.vector.tensor_add(out=x[:, 0:512], in0=psA, in1=decraw[:, t, 0:512])
                nc.vector.tensor_add(
                    out=x[:, 512:1024], in0=psB, in1=decraw[:, t, 512:1024]
                )

                st = stat.tile([128, 2, 6], f32, tag="st")
                nc.vector.bn_stats(out=st[:, 0, :], in_=x[:, 0:512])
                nc.vector.bn_stats(out=st[:, 1, :], in_=x[:, 512:1024])
                mv = stat.tile([128, 2], f32, tag="mv")
                nc.vector.bn_aggr(out=mv, in_=st)
                rs = stat.tile([128, 1], f32, tag="rs")
                nc.scalar.activation(
                    out=rs, in_=mv[:, 1:2],
                    func=mybir.ActivationFunctionType.Sqrt,
                    bias=eps, scale=1.0,
                )
                nc.vector.reciprocal(out=rs, in_=rs)

                o = work.tile([128, D], f32, tag="o", bufs=4)
                nc.vector.tensor_scalar(
                    out=o, in0=x,
                    scalar1=mv[:, 0:1], scalar2=rs,
                    op0=mybir.AluOpType.subtract, op1=mybir.AluOpType.mult,
                )
                if not trivial_affine:
                    nc.vector.tensor_mul(out=o, in0=o, in1=gam_sb)
                    nc.vector.tensor_add(out=o, in0=o, in1=bet_sb)
                nc.scalar.dma_start(out=out_t[t], in_=o)

    _split_excess_waits(nc)
    return nc


def _get_program(add_pos: bool, trivial_affine: bool):
    key = (add_pos, trivial_affine, MM_DTYPE)
    if key not in _PROGS:
        _PROGS[key] = _build_program(add_pos, trivial_affine)
    return _PROGS[key]


def kernel(dec_inp, pos_emb, mems, gamma, beta, add_position):
    from concourse.bass_utils import run_bass_kernel_spmd

    dec_inp = np.asarray(dec_inp, dtype=np.float32)
    pos_emb = np.asarray(pos_emb, dtype=np.float32)
    mems = np.asarray(mems, dtype=np.float32)
    gamma = np.asarray(gamma, dtype=np.float32)
    beta = np.asarray(beta, dtype=np.float32)
    add_pos = bool(int(add_position))
    trivial = bool(np.all(gamma == 1.0) and np.all(beta == 0.0))

    import ml_dtypes

    nc = _get_program(add_pos, trivial)
    wb = _wband().astype(ml_dtypes.bfloat16)
    pos2d = np.ascontiguousarray(pos_emb[:, 0, :])

    in_maps = []
    for b in range(BATCH):
        m = {
            "dec": np.ascontiguousarray(dec_inp[:, b, :]),
            "memsb": np.ascontiguousarray(mems[:, b, :]),
            "wband": wb,
        }
        if add_pos:
            m["pos"] = pos2d
        if not trivial:
            m["gam"] = gamma
            m["bet"] = beta
        in_maps.append(m)

    res = run_bass_kernel_spmd(nc, in_maps, list(range(BATCH)))
    return np.stack([res.results[b]["out"] for b in range(BATCH)], axis=1)


# revision 27
# speedup vs baseline: 1.0936x; 1.0936x over previous
"""Trainium2 Bass kernel for nn_CausalFT (causal Fourier transform + residual + LayerNorm).

reference semantics (QLEN=1024, MLEN=1024, BATCH=8, D_MODEL=1024, klen=2048):
    cat  = concat([mems, dec_inp], axis=0) (+ pos_emb broadcast over batch)
    ft   = einsum('ml,lbd->mbd', ft_matrix(1024, 2048), cat)
    x    = dec_inp + ft / sqrt(2048)
    out  = LayerNorm_d(x) * gamma + beta

Sharding: data-parallel over batch — core b computes out[:, b, :] entirely
(no collectives).  The FT matrix is an input-independent constant, computed
host-side (bit-identical to the reference, via jax on CPU), pre-scaled by
1/sqrt(klen), transposed and band-packed: row-tile t of the output only needs
contraction tiles k in [t, t+8] (the matrix is banded: ft[m, j] != 0 only for
m <= j <= m+1024), which cuts matmul work and weight traffic by ~44%.

Matmuls run in float32r (1 PE cycle/row vs 4 for fp32; measured output
rel-err 7e-6 vs 6e-6 for full fp32 on this problem).
"""

import math

import numpy as np

QLEN, MLEN, BATCH, D = 1024, 1024, 8, 1024
KLEN = QLEN + MLEN
NT = QLEN // 128          # 8 output row tiles
NK = KLEN // 128          # 16 contraction tiles
BW = NK - NT + 1          # 9 band K-tiles per output row tile
LN_EPS = 1e-5

# matmul operand dtype: "f32" (exact, 4 cyc/row) or "f32r" (reduced, 1 cyc/row)
MM_DTYPE = "f32r"

_WBAND = None
_PROGS = {}


def _ft_matrix_np():
    """Replicate reference._ft_matrix bit-for-bit using jax on CPU."""
    import jax
    import jax.numpy as jnp

    cpu = jax.local_devices(backend="cpu")[0]
    with jax.default_device(cpu):
        qlen, klen = QLEN, KLEN
        ft_len = klen - qlen + 1
        m = jnp.arange(qlen, dtype=jnp.float32)
        k = jnp.arange(ft_len, dtype=jnp.float32)
        base = jnp.cos((2.0 * float(np.pi)) * jnp.outer(m, k) / float(ft_len))
        base = base / float(np.sqrt(ft_len))
        mat = jnp.pad(base, ((0, 0), (klen - ft_len, 0)))
        shift = (qlen - 1) - jnp.arange(qlen)
        cols = (jnp.arange(klen)[None, :] + shift[:, None]) % klen
        mat = jnp.take_along_axis(mat, cols, axis=1)
        rows = jnp.arange(qlen)[:, None]
        js = jnp.arange(klen)[None, :]
        mask = (js <= rows + (klen - qlen)) & (js >= rows)
        mat = jnp.where(mask, mat, jnp.float32(0.0))
        return np.asarray(jax.device_get(mat), dtype=np.float32)


def _wband():
    """Band-packed, pre-scaled, transposed FT matrix: [128, NT*BW, 128].

    wband[p, t*BW+i, m] = (W/sqrt(KLEN))[128t+m, 128(t+i)+p]  (lhsT layout).
    """
    global _WBAND
    if _WBAND is None:
        w = _ft_matrix_np() / np.float32(math.sqrt(KLEN))
        wb = np.empty((128, NT * BW, 128), dtype=np.float32)
        for t in range(NT):
            for i in range(BW):
                blk = w[128 * t : 128 * (t + 1), 128 * (t + i) : 128 * (t + i) + 128]
                wb[:, t * BW + i, :] = blk.T
        _WBAND = np.ascontiguousarray(wb)
    return _WBAND


def _install_drain_patch():
    """Work around walrus 'Too many sync wait commands' on the Tile tail drain.

    The stock TileContext._drain_and_barrier emits ONE sync-engine Drain
    carrying a sem wait for every proc lane that ticked (up to 27).  The
    walrus build in this environment accepts only a single sync-wait per
    instruction, so split the global-clock wait set across consecutive
    Drains (one wait each) — sequential execution on the same engine gives
    the same quiescence guarantee.  Also skip the tail per-sem zeroing:
    the bass preamble range-clears every kernel semaphore at program start
    on each execution, so the ~250 walrus-expanded tail EVSEMs (~8us) are
    redundant for re-execution correctness.
    """
    import re

    import bass_rust
    import concourse.tile as _tile
    from concourse.vector_clock import ScopedClock

    if getattr(_tile.TileContext, "_drain_patch_installed", False):
        return

    def _clock_ticks(vc):
        m = re.search(r"\[([0-9, ]*)\]", repr(vc))
        if not m or not m.group(1).strip():
            return []
        return [int(x) for x in m.group(1).split(",")]

    def _patched_drain_and_barrier(self, tick_clock, wait_clock):
        nc = self.nc
        ticks = _clock_ticks(tick_clock.global_clock)
        for i, t in enumerate(ticks):
            if t > 0:
                part = bass_rust.VectorClock()
                part.require_at_least(i, t)
                d = nc.sync.drain()
                wait_clock.add_sem_waits(d.ins, ScopedClock({None: part}))
        assert self.sems is not None
        popped = nc._tile_sem_poison_stack.pop()
        assert popped is self._sem_poison
        nc._state.prepend_free_semaphores(
            [s.num for s in self.sems.allocated().values()]
        )

    _tile.TileContext._drain_and_barrier = _patched_drain_and_barrier
    _tile.TileContext._drain_patch_installed = True


def _split_excess_waits(nc, cap=1):
    """Hoist excess per-instruction sem waits onto preceding same-engine nops.

    The walrus build here accepts only `cap` sync-wait commands per
    instruction.  Engines execute their instruction stream in order, so
    moving waits to immediately-preceding same-engine nops preserves the
    ordering semantics (the instruction still starts only after every wait
    is satisfied).
    """
    import concourse.mybir as mybir

    for bb in nc.main_func.blocks:
        insts = list(bb.instructions)
        if not any(
            i.sync_info and i.sync_info.on_wait and len(i.sync_info.on_wait) > cap
            for i in insts
        ):
            continue
        new = []
        for inst in insts:
            si = inst.sync_info
            waits = list(si.on_wait) if si and si.on_wait else []
            if len(waits) > cap:
                for sw in waits[:-cap]:
                    nop = nc.engines[inst.engine].nop(nofuse=True).ins
                    cur = nc.cur_bb.bb
                    assert cur.instructions and cur.instructions[-1] is nop
                    cur.instructions.pop()
                    nop.sync_info = mybir.SyncInfo(on_wait=[sw], on_update=[])
                    new.append(nop)
                inst.sync_info = mybir.SyncInfo(
                    on_wait=waits[-cap:], on_update=list(si.on_update or [])
                )
            new.append(inst)
        bb.instructions.clear()
        for i in new:
            bb.instructions.append(i)


def _build_program(add_pos: bool, trivial_affine: bool):
    _install_drain_patch()
    import concourse.bass as bass
    import concourse.mybir as mybir
    import concourse.tile as tile

    f32 = mybir.dt.float32
    nc = bass.Bass()

    dec = nc.dram_tensor("dec", [QLEN, D], f32, kind="ExternalInput")
    memsb = nc.dram_tensor("memsb", [MLEN, D], f32, kind="ExternalInput")
    wband = nc.dram_tensor(
        "wband", [128, NT * BW, 128], mybir.dt.bfloat16, kind="ExternalInput"
    )
    pos = None
    if add_pos:
        pos = nc.dram_tensor("pos", [KLEN, D], f32, kind="ExternalInput")
    gam = bet = None
    if not trivial_affine:
        gam = nc.dram_tensor("gam", [D], f32, kind="ExternalInput")
        bet = nc.dram_tensor("bet", [D], f32, kind="ExternalInput")
    out = nc.dram_tensor("out", [QLEN, D], f32, kind="ExternalOutput")

    dec_t = dec.rearrange("(k p) d -> k p d", p=128)      # [8, 128, 1024]
    mems_t = memsb.rearrange("(k p) d -> k p d", p=128)   # [8, 128, 1024]
    out_t = out.rearrange("(t p) d -> t p d", p=128)      # [8, 128, 1024]

    # FP32r: the PE runs 4x faster, but the BIR verifier requires every
    # producer feeding an fp32r matmul to emit fp32r — so type the weight and
    # rhs tiles (and the DMA source APs, a pure bitcast) as float32r.
    mm_dt = f32 if MM_DTYPE == "f32" else mybir.dt.float32r
    src_cast = (lambda ap: ap) if MM_DTYPE == "f32" else (
        lambda ap: ap.bitcast(mybir.dt.float32r)
    )

    with tile.TileContext(nc) as tc:
        with (
            tc.tile_pool(name="big", bufs=1) as big,
            tc.tile_pool(name="posp", bufs=2) as posp,
            tc.tile_pool(name="small", bufs=1) as small,
            tc.tile_pool(name="work", bufs=3) as work,
            tc.tile_pool(name="stat", bufs=4) as stat,
            tc.tile_pool(name="ps", bufs=2, space="PSUM") as ps,
        ):
            eps = small.tile([128, 1], f32)
            nc.vector.memset(eps, LN_EPS)
            gam_sb = bet_sb = None
            if not trivial_affine:
                gam_sb = small.tile([128, D], f32)
                bet_sb = small.tile([128, D], f32)
                gam_ap, bet_ap = gam[:], bet[:]
                nc.sync.dma_start(
                    out=gam_sb,
                    in_=bass.AP(tensor=gam_ap.tensor, offset=0, ap=[[0, 128]] + list(gam_ap.ap)),
                )
                nc.sync.dma_start(
                    out=bet_sb,
                    in_=bass.AP(tensor=bet_ap.tensor, offset=0, ap=[[0, 128]] + list(bet_ap.ap)),
                )

            # --- weights: band-packed lhsT, shipped as bf16 (half the HBM
            # bytes; the rounding is ~2e-5 on the output since W carries a
            # 1/sqrt(1025*2048) scale) and up-converted to f32r on GpSimd.
            # Chunk t == the 9 tiles of matmul group t, so conversion order
            # matches consumption order.  t=0 first, rest after mems/pos. ---
            wb = big.tile([128, NT * BW, 128], mm_dt)

            def wb_chunk(t):
                wst = posp.tile([128, BW, 128], mybir.dt.bfloat16, tag="wst", bufs=2)
                nc.gpsimd.dma_start(out=wst, in_=wband[:, BW * t : BW * (t + 1), :])
                nc.scalar.copy(out=wb[:, BW * t : BW * (t + 1), :], in_=wst)

            wb_chunk(0)

            # --- rhs: catp[k] = cat tile k (+ pos), decraw for the residual.
            # Interleave mems/pos loads and the pos-adds per k so low-k catp
            # tiles (which gate the first matmul groups) complete earliest.
            catp = big.tile([128, NK, D], mm_dt)
            decraw = big.tile([128, NT, D], f32)
            pos_k = pos.rearrange("(k p) d -> k p d", p=128) if add_pos else None

            def pos_add(k, in0):
                # GpSimd takes every third add (2x slower than DVE but runs
                # in parallel; the add chain paces the matmul groups)
                pc = posp.tile([128, D], f32, tag="pos", bufs=4)
                nc.scalar.dma_start(out=pc, in_=pos_k[k])
                eng = nc.gpsimd if k % 3 == 2 else nc.vector
                eng.tensor_add(out=catp[:, k, :], in0=in0, in1=pc)

            for k in range(8):
                nc.sync.dma_start(out=catp[:, k, :], in_=src_cast(mems_t[k]))
                if add_pos:
                    pos_add(k, catp[:, k, :])
            nc.sync.dma_start(out=decraw[:, 0, :], in_=dec_t[0])
            if add_pos:
                pos_add(8, decraw[:, 0, :])
            else:
                nc.gpsimd.tensor_copy(out=catp[:, 8, :], in_=decraw[:, 0, :])
            for t in (1, 2, 3, 4):
                wb_chunk(t)
            for k in range(9, 16):
                nc.sync.dma_start(out=decraw[:, k - 8, :], in_=dec_t[k - 8])
                if add_pos:
                    pos_add(k, decraw[:, k - 8, :])
                else:
                    nc.gpsimd.tensor_copy(out=catp[:, k, :], in_=decraw[:, k - 8, :])
            for t in (5, 6, 7):
                wb_chunk(t)

            # --- band matmul + fused residual/LayerNorm epilogue per row tile ---
            for t in range(NT):
                psA = ps.tile([128, 512], f32, tag="A", bufs=4)
                psB = ps.tile([128, 512], f32, tag="B", bufs=4)
                for i in range(BW):
                    k = t + i
                    wt = wb[:, t * BW + i, :]
                    nc.tensor.matmul(
                        psA, wt, catp[:, k, 0:512], start=(i == 0), stop=(i == BW - 1)
                    )
                    nc.tensor.matmul(
                        psB, wt, catp[:, k, 512:1024], start=(i == 0), stop=(i == BW - 1)
                    )

                x = work.tile([128, D], f32, tag="x", bufs=4)
                nc.vector.tensor_add(out=x[:, 0:512], in0=psA, in1=decraw[:, t, 0:512])
                nc.vector.tensor_add(
                    out=x[:, 512:1024], in0=psB, in1=decraw[:, t, 512:1024]
                )

                st = stat.tile([128, 2, 6], f32, tag="st")
                nc.vector.bn_stats(out=st[:, 0, :], in_=x[:, 0:512])
                nc.vector.bn_stats(out=st[:, 1, :], in_=x[:, 512:1024])
                mv = stat.tile([128, 2], f32, tag="mv")
                nc.vector.bn_aggr(out=mv, in_=st)
                rs = stat.tile([128, 1], f32, tag="rs")
                nc.scalar.activation(
                    out=rs, in_=mv[:, 1:2],
                    func=mybir.ActivationFunctionType.Sqrt,
                    bias=eps, scale=1.0,
                )
                nc.vector.reciprocal(out=rs, in_=rs)

                o = work.tile([128, D], f32, tag="o", bufs=4)
                nc.vector.tensor_scalar(
                    out=o, in0=x,
                    scalar1=mv[:, 0:1], scalar2=rs,
                    op0=mybir.AluOpType.subtract, op1=mybir.AluOpType.mult,
                )
                if not trivial_affine:
                    nc.vector.tensor_mul(out=o, in0=o, in1=gam_sb)
                    nc.vector.tensor_add(out=o, in0=o, in1=bet_sb)
                nc.scalar.dma_start(out=out_t[t], in_=o)

    _split_excess_waits(nc)
    return nc


def _get_program(add_pos: bool, trivial_affine: bool):
    key = (add_pos, trivial_affine, MM_DTYPE)
    if key not in _PROGS:
        _PROGS[key] = _build_program(add_pos, trivial_affine)
    return _PROGS[key]


def kernel(dec_inp, pos_emb, mems, gamma, beta, add_position):
    from concourse.bass_utils import run_bass_kernel_spmd

    dec_inp = np.asarray(dec_inp, dtype=np.float32)
    pos_emb = np.asarray(pos_emb, dtype=np.float32)
    mems = np.asarray(mems, dtype=np.float32)
    gamma = np.asarray(gamma, dtype=np.float32)
    beta = np.asarray(beta, dtype=np.float32)
    add_pos = bool(int(add_position))
    trivial = bool(np.all(gamma == 1.0) and np.all(beta == 0.0))

    import ml_dtypes

    nc = _get_program(add_pos, trivial)
    wb = _wband().astype(ml_dtypes.bfloat16)
    pos2d = np.ascontiguousarray(pos_emb[:, 0, :])

    in_maps = []
    for b in range(BATCH):
        m = {
            "dec": np.ascontiguousarray(dec_inp[:, b, :]),
            "memsb": np.ascontiguousarray(mems[:, b, :]),
            "wband": wb,
        }
        if add_pos:
            m["pos"] = pos2d
        if not trivial:
            m["gam"] = gamma
            m["bet"] = beta
        in_maps.append(m)

    res = run_bass_kernel_spmd(nc, in_maps, list(range(BATCH)))
    return np.stack([res.results[b]["out"] for b in range(BATCH)], axis=1)


# revision 28
# speedup vs baseline: 1.0940x; 1.0003x over previous
"""Trainium2 Bass kernel for nn_CausalFT (causal Fourier transform + residual + LayerNorm).

reference semantics (QLEN=1024, MLEN=1024, BATCH=8, D_MODEL=1024, klen=2048):
    cat  = concat([mems, dec_inp], axis=0) (+ pos_emb broadcast over batch)
    ft   = einsum('ml,lbd->mbd', ft_matrix(1024, 2048), cat)
    x    = dec_inp + ft / sqrt(2048)
    out  = LayerNorm_d(x) * gamma + beta

Sharding: data-parallel over batch — core b computes out[:, b, :] entirely
(no collectives).  The FT matrix is an input-independent constant, computed
host-side (bit-identical to the reference, via jax on CPU), pre-scaled by
1/sqrt(klen), transposed and band-packed: row-tile t of the output only needs
contraction tiles k in [t, t+8] (the matrix is banded: ft[m, j] != 0 only for
m <= j <= m+1024), which cuts matmul work and weight traffic by ~44%.

Matmuls run in float32r (1 PE cycle/row vs 4 for fp32; measured output
rel-err 7e-6 vs 6e-6 for full fp32 on this problem).
"""

import math

import numpy as np

QLEN, MLEN, BATCH, D = 1024, 1024, 8, 1024
KLEN = QLEN + MLEN
NT = QLEN // 128          # 8 output row tiles
NK = KLEN // 128          # 16 contraction tiles
BW = NK - NT + 1          # 9 band K-tiles per output row tile
LN_EPS = 1e-5

# matmul operand dtype: "f32" (exact, 4 cyc/row) or "f32r" (reduced, 1 cyc/row)
MM_DTYPE = "f32r"

_WBAND = None
_PROGS = {}


def _ft_matrix_np():
    """Replicate reference._ft_matrix bit-for-bit using jax on CPU."""
    import jax
    import jax.numpy as jnp

    cpu = jax.local_devices(backend="cpu")[0]
    with jax.default_device(cpu):
        qlen, klen = QLEN, KLEN
        ft_len = klen - qlen + 1
        m = jnp.arange(qlen, dtype=jnp.float32)
        k = jnp.arange(ft_len, dtype=jnp.float32)
        base = jnp.cos((2.0 * float(np.pi)) * jnp.outer(m, k) / float(ft_len))
        base = base / float(np.sqrt(ft_len))
        mat = jnp.pad(base, ((0, 0), (klen - ft_len, 0)))
        shift = (qlen - 1) - jnp.arange(qlen)
        cols = (jnp.arange(klen)[None, :] + shift[:, None]) % klen
        mat = jnp.take_along_axis(mat, cols, axis=1)
        rows = jnp.arange(qlen)[:, None]
        js = jnp.arange(klen)[None, :]
        mask = (js <= rows + (klen - qlen)) & (js >= rows)
        mat = jnp.where(mask, mat, jnp.float32(0.0))
        return np.asarray(jax.device_get(mat), dtype=np.float32)


def _wband():
    """Band-packed, pre-scaled, transposed FT matrix: [128, NT*BW, 128].

    wband[p, t*BW+i, m] = (W/sqrt(KLEN))[128t+m, 128(t+i)+p]  (lhsT layout).
    """
    global _WBAND
    if _WBAND is None:
        w = _ft_matrix_np() / np.float32(math.sqrt(KLEN))
        wb = np.empty((128, NT * BW, 128), dtype=np.float32)
        for t in range(NT):
            for i in range(BW):
                blk = w[128 * t : 128 * (t + 1), 128 * (t + i) : 128 * (t + i) + 128]
                wb[:, t * BW + i, :] = blk.T
        _WBAND = np.ascontiguousarray(wb)
    return _WBAND


def _install_drain_patch():
    """Work around walrus 'Too many sync wait commands' on the Tile tail drain.

    The stock TileContext._drain_and_barrier emits ONE sync-engine Drain
    carrying a sem wait for every proc lane that ticked (up to 27).  The
    walrus build in this environment accepts only a single sync-wait per
    instruction, so split the global-clock wait set across consecutive
    Drains (one wait each) — sequential execution on the same engine gives
    the same quiescence guarantee.  Also skip the tail per-sem zeroing:
    the bass preamble range-clears every kernel semaphore at program start
    on each execution, so the ~250 walrus-expanded tail EVSEMs (~8us) are
    redundant for re-execution correctness.
    """
    import re

    import bass_rust
    import concourse.tile as _tile
    from concourse.vector_clock import ScopedClock

    if getattr(_tile.TileContext, "_drain_patch_installed", False):
        return

    def _clock_ticks(vc):
        m = re.search(r"\[([0-9, ]*)\]", repr(vc))
        if not m or not m.group(1).strip():
            return []
        return [int(x) for x in m.group(1).split(",")]

    def _patched_drain_and_barrier(self, tick_clock, wait_clock):
        nc = self.nc
        ticks = _clock_ticks(tick_clock.global_clock)
        for i, t in enumerate(ticks):
            if t > 0:
                part = bass_rust.VectorClock()
                part.require_at_least(i, t)
                d = nc.sync.drain()
                wait_clock.add_sem_waits(d.ins, ScopedClock({None: part}))
        assert self.sems is not None
        popped = nc._tile_sem_poison_stack.pop()
        assert popped is self._sem_poison
        nc._state.prepend_free_semaphores(
            [s.num for s in self.sems.allocated().values()]
        )

    _tile.TileContext._drain_and_barrier = _patched_drain_and_barrier
    _tile.TileContext._drain_patch_installed = True


def _split_excess_waits(nc, cap=1):
    """Hoist excess per-instruction sem waits onto preceding same-engine nops.

    The walrus build here accepts only `cap` sync-wait commands per
    instruction.  Engines execute their instruction stream in order, so
    moving waits to immediately-preceding same-engine nops preserves the
    ordering semantics (the instruction still starts only after every wait
    is satisfied).
    """
    import concourse.mybir as mybir

    for bb in nc.main_func.blocks:
        insts = list(bb.instructions)
        if not any(
            i.sync_info and i.sync_info.on_wait and len(i.sync_info.on_wait) > cap
            for i in insts
        ):
            continue
        new = []
        for inst in insts:
            si = inst.sync_info
            waits = list(si.on_wait) if si and si.on_wait else []
            if len(waits) > cap:
                for sw in waits[:-cap]:
                    nop = nc.engines[inst.engine].nop(nofuse=True).ins
                    cur = nc.cur_bb.bb
                    assert cur.instructions and cur.instructions[-1] is nop
                    cur.instructions.pop()
                    nop.sync_info = mybir.SyncInfo(on_wait=[sw], on_update=[])
                    new.append(nop)
                inst.sync_info = mybir.SyncInfo(
                    on_wait=waits[-cap:], on_update=list(si.on_update or [])
                )
            new.append(inst)
        bb.instructions.clear()
        for i in new:
            bb.instructions.append(i)


def _build_program(add_pos: bool, trivial_affine: bool):
    _install_drain_patch()
    import concourse.bass as bass
    import concourse.mybir as mybir
    import concourse.tile as tile

    f32 = mybir.dt.float32
    nc = bass.Bass()

    dec = nc.dram_tensor("dec", [QLEN, D], f32, kind="ExternalInput")
    memsb = nc.dram_tensor("memsb", [MLEN, D], f32, kind="ExternalInput")
    wband = nc.dram_tensor(
        "wband", [128, NT * BW, 128], mybir.dt.bfloat16, kind="ExternalInput"
    )
    pos = None
    if add_pos:
        pos = nc.dram_tensor("pos", [KLEN, D], f32, kind="ExternalInput")
    gam = bet = None
    if not trivial_affine:
        gam = nc.dram_tensor("gam", [D], f32, kind="ExternalInput")
        bet = nc.dram_tensor("bet", [D], f32, kind="ExternalInput")
    out = nc.dram_tensor("out", [QLEN, D], f32, kind="ExternalOutput")

    dec_t = dec.rearrange("(k p) d -> k p d", p=128)      # [8, 128, 1024]
    mems_t = memsb.rearrange("(k p) d -> k p d", p=128)   # [8, 128, 1024]
    out_t = out.rearrange("(t p) d -> t p d", p=128)      # [8, 128, 1024]

    # FP32r: the PE runs 4x faster, but the BIR verifier requires every
    # producer feeding an fp32r matmul to emit fp32r — so type the weight and
    # rhs tiles (and the DMA source APs, a pure bitcast) as float32r.
    mm_dt = f32 if MM_DTYPE == "f32" else mybir.dt.float32r
    src_cast = (lambda ap: ap) if MM_DTYPE == "f32" else (
        lambda ap: ap.bitcast(mybir.dt.float32r)
    )

    with tile.TileContext(nc) as tc:
        with (
            tc.tile_pool(name="big", bufs=1) as big,
            tc.tile_pool(name="posp", bufs=2) as posp,
            tc.tile_pool(name="small", bufs=1) as small,
            tc.tile_pool(name="work", bufs=3) as work,
            tc.tile_pool(name="stat", bufs=4) as stat,
            tc.tile_pool(name="ps", bufs=2, space="PSUM") as ps,
        ):
            eps = small.tile([128, 1], f32)
            nc.vector.memset(eps, LN_EPS)
            gam_sb = bet_sb = None
            if not trivial_affine:
                gam_sb = small.tile([128, D], f32)
                bet_sb = small.tile([128, D], f32)
                gam_ap, bet_ap = gam[:], bet[:]
                nc.sync.dma_start(
                    out=gam_sb,
                    in_=bass.AP(tensor=gam_ap.tensor, offset=0, ap=[[0, 128]] + list(gam_ap.ap)),
                )
                nc.sync.dma_start(
                    out=bet_sb,
                    in_=bass.AP(tensor=bet_ap.tensor, offset=0, ap=[[0, 128]] + list(bet_ap.ap)),
                )

            # --- weights: band-packed lhsT, shipped as bf16 (half the HBM
            # bytes; the rounding is ~2e-5 on the output since W carries a
            # 1/sqrt(1025*2048) scale) and up-converted to f32r on GpSimd.
            # Chunk t == the 9 tiles of matmul group t, so conversion order
            # matches consumption order.  t=0 first, rest after mems/pos. ---
            wb = big.tile([128, NT * BW, 128], mm_dt)

            def wb_chunk(t):
                wst = posp.tile([128, BW, 128], mybir.dt.bfloat16, tag="wst", bufs=2)
                nc.gpsimd.dma_start(out=wst, in_=wband[:, BW * t : BW * (t + 1), :])
                nc.scalar.copy(out=wb[:, BW * t : BW * (t + 1), :], in_=wst)

            wb_chunk(0)

            # --- rhs: catp[k] = cat tile k (+ pos), decraw for the residual.
            # Interleave mems/pos loads and the pos-adds per k so low-k catp
            # tiles (which gate the first matmul groups) complete earliest.
            # catp as a sliding window: tile k is dead once group k's matmuls
            # finish, so 12 rotating slots suffice (9-tile band + margin);
            # the freed SBUF holds all 8 output tiles so stores can be
            # emitted last and stop stealing HBM bandwidth from the input
            # stream mid-kernel.
            catp_t = []
            decraw = big.tile([128, NT, D], f32)
            pos_k = pos.rearrange("(k p) d -> k p d", p=128) if add_pos else None

            def new_ck(k):
                ck = work.tile([128, D], mm_dt, tag="ck", bufs=12, name=f"ck{k}")
                catp_t.append(ck)
                return ck

            def pos_add(k, ck, in0):
                # GpSimd takes every third add (2x slower than DVE but runs
                # in parallel; the add chain paces the matmul groups)
                pc = posp.tile([128, D], f32, tag="pos", bufs=4)
                nc.scalar.dma_start(out=pc, in_=pos_k[k])
                eng = nc.gpsimd if k % 3 == 2 else nc.vector
                eng.tensor_add(out=ck, in0=in0, in1=pc)

            for k in range(8):
                ck = new_ck(k)
                nc.sync.dma_start(out=ck, in_=src_cast(mems_t[k]))
                if add_pos:
                    pos_add(k, ck, ck)
            ck = new_ck(8)
            nc.sync.dma_start(out=decraw[:, 0, :], in_=dec_t[0])
            if add_pos:
                pos_add(8, ck, decraw[:, 0, :])
            else:
                nc.gpsimd.tensor_copy(out=ck, in_=decraw[:, 0, :])
            for t in (1, 2, 3, 4):
                wb_chunk(t)
            for k in range(9, 16):
                ck = new_ck(k)
                nc.sync.dma_start(out=decraw[:, k - 8, :], in_=dec_t[k - 8])
                if add_pos:
                    pos_add(k, ck, decraw[:, k - 8, :])
                else:
                    nc.gpsimd.tensor_copy(out=ck, in_=decraw[:, k - 8, :])
            for t in (5, 6, 7):
                wb_chunk(t)

            # --- band matmul + fused residual/LayerNorm epilogue per row tile ---
            out_tiles = []
            for t in range(NT):
                psA = ps.tile([128, 512], f32, tag="A", bufs=4)
                psB = ps.tile([128, 512], f32, tag="B", bufs=4)
                for i in range(BW):
                    k = t + i
                    wt = wb[:, t * BW + i, :]
                    nc.tensor.matmul(
                        psA, wt, catp_t[k][:, 0:512], start=(i == 0), stop=(i == BW - 1)
                    )
                    nc.tensor.matmul(
                        psB, wt, catp_t[k][:, 512:1024], start=(i == 0), stop=(i == BW - 1)
                    )

                x = work.tile([128, D], f32, tag="x", bufs=4)
                nc.vector.tensor_add(out=x[:, 0:512], in0=psA, in1=decraw[:, t, 0:512])
                nc.vector.tensor_add(
                    out=x[:, 512:1024], in0=psB, in1=decraw[:, t, 512:1024]
                )

                st = stat.tile([128, 2, 6], f32, tag="st")
                nc.vector.bn_stats(out=st[:, 0, :], in_=x[:, 0:512])
                nc.vector.bn_stats(out=st[:, 1, :], in_=x[:, 512:1024])
                mv = stat.tile([128, 2], f32, tag="mv")
                nc.vector.bn_aggr(out=mv, in_=st)
                rs = stat.tile([128, 1], f32, tag="rs")
                nc.scalar.activation(
                    out=rs, in_=mv[:, 1:2],
                    func=mybir.ActivationFunctionType.Sqrt,
                    bias=eps, scale=1.0,
                )
                nc.vector.reciprocal(out=rs, in_=rs)

                o = work.tile([128, D], f32, tag="o", bufs=8, name=f"o{t}")
                nc.vector.tensor_scalar(
                    out=o, in0=x,
                    scalar1=mv[:, 0:1], scalar2=rs,
                    op0=mybir.AluOpType.subtract, op1=mybir.AluOpType.mult,
                )
                if not trivial_affine:
                    nc.vector.tensor_mul(out=o, in0=o, in1=gam_sb)
                    nc.vector.tensor_add(out=o, in0=o, in1=bet_sb)
                out_tiles.append((t, o))

            # stores last: they queue behind the input stream instead of
            # competing with it for HBM bandwidth mid-kernel
            for t, o in out_tiles:
                nc.scalar.dma_start(out=out_t[t], in_=o)

    _split_excess_waits(nc)
    return nc


def _get_program(add_pos: bool, trivial_affine: bool):
    key = (add_pos, trivial_affine, MM_DTYPE)
    if key not in _PROGS:
        _PROGS[key] = _build_program(add_pos, trivial_affine)
    return _PROGS[key]


def kernel(dec_inp, pos_emb, mems, gamma, beta, add_position):
    from concourse.bass_utils import run_bass_kernel_spmd

    dec_inp = np.asarray(dec_inp, dtype=np.float32)
    pos_emb = np.asarray(pos_emb, dtype=np.float32)
    mems = np.asarray(mems, dtype=np.float32)
    gamma = np.asarray(gamma, dtype=np.float32)
    beta = np.asarray(beta, dtype=np.float32)
    add_pos = bool(int(add_position))
    trivial = bool(np.all(gamma == 1.0) and np.all(beta == 0.0))

    import ml_dtypes

    nc = _get_program(add_pos, trivial)
    wb = _wband().astype(ml_dtypes.bfloat16)
    pos2d = np.ascontiguousarray(pos_emb[:, 0, :])

    in_maps = []
    for b in range(BATCH):
        m = {
            "dec": np.ascontiguousarray(dec_inp[:, b, :]),
            "memsb": np.ascontiguousarray(mems[:, b, :]),
            "wband": wb,
        }
        if add_pos:
            m["pos"] = pos2d
        if not trivial:
            m["gam"] = gamma
            m["bet"] = beta
        in_maps.append(m)

    res = run_bass_kernel_spmd(nc, in_maps, list(range(BATCH)))
    return np.stack([res.results[b]["out"] for b in range(BATCH)], axis=1)


# revision 29
# speedup vs baseline: 1.0978x; 1.0035x over previous
"""Trainium2 Bass kernel for nn_CausalFT (causal Fourier transform + residual + LayerNorm).

reference semantics (QLEN=1024, MLEN=1024, BATCH=8, D_MODEL=1024, klen=2048):
    cat  = concat([mems, dec_inp], axis=0) (+ pos_emb broadcast over batch)
    ft   = einsum('ml,lbd->mbd', ft_matrix(1024, 2048), cat)
    x    = dec_inp + ft / sqrt(2048)
    out  = LayerNorm_d(x) * gamma + beta

Sharding: data-parallel over batch — core b computes out[:, b, :] entirely
(no collectives).  The FT matrix is an input-independent constant, computed
host-side (bit-identical to the reference, via jax on CPU), pre-scaled by
1/sqrt(klen), transposed and band-packed: row-tile t of the output only needs
contraction tiles k in [t, t+8] (the matrix is banded: ft[m, j] != 0 only for
m <= j <= m+1024), which cuts matmul work and weight traffic by ~44%.

Matmuls run in float32r (1 PE cycle/row vs 4 for fp32; measured output
rel-err 7e-6 vs 6e-6 for full fp32 on this problem).
"""

import math

import numpy as np

QLEN, MLEN, BATCH, D = 1024, 1024, 8, 1024
KLEN = QLEN + MLEN
NT = QLEN // 128          # 8 output row tiles
NK = KLEN // 128          # 16 contraction tiles
BW = NK - NT + 1          # 9 band K-tiles per output row tile
LN_EPS = 1e-5

# matmul operand dtype: "f32" (exact, 4 cyc/row) or "f32r" (reduced, 1 cyc/row)
MM_DTYPE = "f32r"

_WBAND = None
_PROGS = {}


def _ft_matrix_np():
    """Replicate reference._ft_matrix bit-for-bit using jax on CPU."""
    import jax
    import jax.numpy as jnp

    cpu = jax.local_devices(backend="cpu")[0]
    with jax.default_device(cpu):
        qlen, klen = QLEN, KLEN
        ft_len = klen - qlen + 1
        m = jnp.arange(qlen, dtype=jnp.float32)
        k = jnp.arange(ft_len, dtype=jnp.float32)
        base = jnp.cos((2.0 * float(np.pi)) * jnp.outer(m, k) / float(ft_len))
        base = base / float(np.sqrt(ft_len))
        mat = jnp.pad(base, ((0, 0), (klen - ft_len, 0)))
        shift = (qlen - 1) - jnp.arange(qlen)
        cols = (jnp.arange(klen)[None, :] + shift[:, None]) % klen
        mat = jnp.take_along_axis(mat, cols, axis=1)
        rows = jnp.arange(qlen)[:, None]
        js = jnp.arange(klen)[None, :]
        mask = (js <= rows + (klen - qlen)) & (js >= rows)
        mat = jnp.where(mask, mat, jnp.float32(0.0))
        return np.asarray(jax.device_get(mat), dtype=np.float32)


def _wband():
    """Band-packed, pre-scaled, transposed FT matrix: [128, NT*BW, 128].

    wband[p, t*BW+i, m] = (W/sqrt(KLEN))[128t+m, 128(t+i)+p]  (lhsT layout).
    """
    global _WBAND
    if _WBAND is None:
        w = _ft_matrix_np() / np.float32(math.sqrt(KLEN))
        wb = np.empty((128, NT * BW, 128), dtype=np.float32)
        for t in range(NT):
            for i in range(BW):
                blk = w[128 * t : 128 * (t + 1), 128 * (t + i) : 128 * (t + i) + 128]
                wb[:, t * BW + i, :] = blk.T
        _WBAND = np.ascontiguousarray(wb)
    return _WBAND


def _install_drain_patch():
    """Work around walrus 'Too many sync wait commands' on the Tile tail drain.

    The stock TileContext._drain_and_barrier emits ONE sync-engine Drain
    carrying a sem wait for every proc lane that ticked (up to 27).  The
    walrus build in this environment accepts only a single sync-wait per
    instruction, so split the global-clock wait set across consecutive
    Drains (one wait each) — sequential execution on the same engine gives
    the same quiescence guarantee.  Also skip the tail per-sem zeroing:
    the bass preamble range-clears every kernel semaphore at program start
    on each execution, so the ~250 walrus-expanded tail EVSEMs (~8us) are
    redundant for re-execution correctness.
    """
    import re

    import bass_rust
    import concourse.tile as _tile
    from concourse.vector_clock import ScopedClock

    if getattr(_tile.TileContext, "_drain_patch_installed", False):
        return

    def _clock_ticks(vc):
        m = re.search(r"\[([0-9, ]*)\]", repr(vc))
        if not m or not m.group(1).strip():
            return []
        return [int(x) for x in m.group(1).split(",")]

    def _patched_drain_and_barrier(self, tick_clock, wait_clock):
        nc = self.nc
        ticks = _clock_ticks(tick_clock.global_clock)
        for i, t in enumerate(ticks):
            if t > 0:
                part = bass_rust.VectorClock()
                part.require_at_least(i, t)
                d = nc.sync.drain()
                wait_clock.add_sem_waits(d.ins, ScopedClock({None: part}))
        assert self.sems is not None
        popped = nc._tile_sem_poison_stack.pop()
        assert popped is self._sem_poison
        nc._state.prepend_free_semaphores(
            [s.num for s in self.sems.allocated().values()]
        )

    _tile.TileContext._drain_and_barrier = _patched_drain_and_barrier
    _tile.TileContext._drain_patch_installed = True


def _split_excess_waits(nc, cap=1):
    """Hoist excess per-instruction sem waits onto preceding same-engine nops.

    The walrus build here accepts only `cap` sync-wait commands per
    instruction.  Engines execute their instruction stream in order, so
    moving waits to immediately-preceding same-engine nops preserves the
    ordering semantics (the instruction still starts only after every wait
    is satisfied).
    """
    import concourse.mybir as mybir

    for bb in nc.main_func.blocks:
        insts = list(bb.instructions)
        if not any(
            i.sync_info and i.sync_info.on_wait and len(i.sync_info.on_wait) > cap
            for i in insts
        ):
            continue
        new = []
        for inst in insts:
            si = inst.sync_info
            waits = list(si.on_wait) if si and si.on_wait else []
            if len(waits) > cap:
                for sw in waits[:-cap]:
                    nop = nc.engines[inst.engine].nop(nofuse=True).ins
                    cur = nc.cur_bb.bb
                    assert cur.instructions and cur.instructions[-1] is nop
                    cur.instructions.pop()
                    nop.sync_info = mybir.SyncInfo(on_wait=[sw], on_update=[])
                    new.append(nop)
                inst.sync_info = mybir.SyncInfo(
                    on_wait=waits[-cap:], on_update=list(si.on_update or [])
                )
            new.append(inst)
        bb.instructions.clear()
        for i in new:
            bb.instructions.append(i)


def _build_program(add_pos: bool, trivial_affine: bool):
    _install_drain_patch()
    import concourse.bass as bass
    import concourse.mybir as mybir
    import concourse.tile as tile

    f32 = mybir.dt.float32
    nc = bass.Bass()

    dec = nc.dram_tensor("dec", [QLEN, D], f32, kind="ExternalInput")
    memsb = nc.dram_tensor("memsb", [MLEN, D], f32, kind="ExternalInput")
    wband = nc.dram_tensor(
        "wband", [128, NT * BW, 128], mybir.dt.bfloat16, kind="ExternalInput"
    )
    pos = None
    if add_pos:
        pos = nc.dram_tensor("pos", [KLEN, D], f32, kind="ExternalInput")
    gam = bet = None
    if not trivial_affine:
        gam = nc.dram_tensor("gam", [D], f32, kind="ExternalInput")
        bet = nc.dram_tensor("bet", [D], f32, kind="ExternalInput")
    out = nc.dram_tensor("out", [QLEN, D], f32, kind="ExternalOutput")

    dec_t = dec.rearrange("(k p) d -> k p d", p=128)      # [8, 128, 1024]
    mems_t = memsb.rearrange("(k p) d -> k p d", p=128)   # [8, 128, 1024]
    out_t = out.rearrange("(t p) d -> t p d", p=128)      # [8, 128, 1024]

    # FP32r: the PE runs 4x faster, but the BIR verifier requires every
    # producer feeding an fp32r matmul to emit fp32r — so type the weight and
    # rhs tiles (and the DMA source APs, a pure bitcast) as float32r.
    mm_dt = f32 if MM_DTYPE == "f32" else mybir.dt.float32r
    src_cast = (lambda ap: ap) if MM_DTYPE == "f32" else (
        lambda ap: ap.bitcast(mybir.dt.float32r)
    )

    with tile.TileContext(nc) as tc:
        with (
            tc.tile_pool(name="big", bufs=1) as big,
            tc.tile_pool(name="posp", bufs=2) as posp,
            tc.tile_pool(name="small", bufs=1) as small,
            tc.tile_pool(name="work", bufs=3) as work,
            tc.tile_pool(name="stat", bufs=4) as stat,
            tc.tile_pool(name="ps", bufs=2, space="PSUM") as ps,
        ):
            eps = small.tile([128, 1], f32)
            nc.vector.memset(eps, LN_EPS)
            gam_sb = bet_sb = None
            if not trivial_affine:
                gam_sb = small.tile([128, D], f32)
                bet_sb = small.tile([128, D], f32)
                gam_ap, bet_ap = gam[:], bet[:]
                nc.sync.dma_start(
                    out=gam_sb,
                    in_=bass.AP(tensor=gam_ap.tensor, offset=0, ap=[[0, 128]] + list(gam_ap.ap)),
                )
                nc.sync.dma_start(
                    out=bet_sb,
                    in_=bass.AP(tensor=bet_ap.tensor, offset=0, ap=[[0, 128]] + list(bet_ap.ap)),
                )

            # --- weights: band-packed lhsT, shipped as bf16 (half the HBM
            # bytes; the rounding is ~2e-5 on the output since W carries a
            # 1/sqrt(1025*2048) scale) and up-converted to f32r on GpSimd.
            # Chunk t == the 9 tiles of matmul group t, so conversion order
            # matches consumption order.  t=0 first, rest after mems/pos. ---
            wb = big.tile([128, NT * BW, 128], mm_dt)

            def wb_chunk(t):
                wst = posp.tile([128, BW, 128], mybir.dt.bfloat16, tag="wst", bufs=2)
                nc.gpsimd.dma_start(out=wst, in_=wband[:, BW * t : BW * (t + 1), :])
                nc.scalar.copy(out=wb[:, BW * t : BW * (t + 1), :], in_=wst)

            wb_chunk(0)

            # --- rhs: catp[k] = cat tile k (+ pos), decraw for the residual.
            # Interleave mems/pos loads and the pos-adds per k so low-k catp
            # tiles (which gate the first matmul groups) complete earliest.
            # catp as a sliding window: tile k is dead once group k's matmuls
            # finish, so 12 rotating slots suffice (9-tile band + margin);
            # the freed SBUF holds all 8 output tiles so stores can be
            # emitted last and stop stealing HBM bandwidth from the input
            # stream mid-kernel.
            catp_t = []
            decraw = big.tile([128, NT, D], f32)
            pos_k = pos.rearrange("(k p) d -> k p d", p=128) if add_pos else None

            def new_ck(k):
                ck = work.tile([128, D], mm_dt, tag="ck", bufs=12, name=f"ck{k}")
                catp_t.append(ck)
                return ck

            def pos_add(k, ck, in0):
                # GpSimd takes every third add (2x slower than DVE but runs
                # in parallel; the add chain paces the matmul groups)
                pc = posp.tile([128, D], f32, tag="pos", bufs=4)
                nc.scalar.dma_start(out=pc, in_=pos_k[k])
                eng = nc.gpsimd if k % 3 == 2 else nc.vector
                eng.tensor_add(out=ck, in0=in0, in1=pc)

            for k in range(8):
                ck = new_ck(k)
                nc.sync.dma_start(out=ck, in_=src_cast(mems_t[k]))
                if add_pos:
                    pos_add(k, ck, ck)
            ck = new_ck(8)
            nc.sync.dma_start(out=decraw[:, 0, :], in_=dec_t[0])
            if add_pos:
                pos_add(8, ck, decraw[:, 0, :])
            else:
                nc.gpsimd.tensor_copy(out=ck, in_=decraw[:, 0, :])
            for t in (1, 2, 3, 4):
                wb_chunk(t)
            for k in range(9, 16):
                ck = new_ck(k)
                nc.sync.dma_start(out=decraw[:, k - 8, :], in_=dec_t[k - 8])
                if add_pos:
                    pos_add(k, ck, decraw[:, k - 8, :])
                else:
                    nc.gpsimd.tensor_copy(out=ck, in_=decraw[:, k - 8, :])
            for t in (5, 6, 7):
                wb_chunk(t)

            # --- band matmul + fused residual/LayerNorm epilogue per row tile ---
            out_tiles = []
            for t in range(NT):
                psA = ps.tile([128, 512], f32, tag="A", bufs=4)
                psB = ps.tile([128, 512], f32, tag="B", bufs=4)
                for i in range(BW):
                    k = t + i
                    wt = wb[:, t * BW + i, :]
                    nc.tensor.matmul(
                        psA, wt, catp_t[k][:, 0:512], start=(i == 0), stop=(i == BW - 1)
                    )
                    nc.tensor.matmul(
                        psB, wt, catp_t[k][:, 512:1024], start=(i == 0), stop=(i == BW - 1)
                    )

                x = work.tile([128, D], f32, tag="x", bufs=4)
                nc.vector.tensor_add(out=x[:, 0:512], in0=psA, in1=decraw[:, t, 0:512])
                nc.vector.tensor_add(
                    out=x[:, 512:1024], in0=psB, in1=decraw[:, t, 512:1024]
                )

                st = stat.tile([128, 2, 6], f32, tag="st")
                nc.vector.bn_stats(out=st[:, 0, :], in_=x[:, 0:512])
                nc.vector.bn_stats(out=st[:, 1, :], in_=x[:, 512:1024])
                mv = stat.tile([128, 2], f32, tag="mv")
                nc.vector.bn_aggr(out=mv, in_=st)
                rs = stat.tile([128, 1], f32, tag="rs")
                nc.scalar.activation(
                    out=rs, in_=mv[:, 1:2],
                    func=mybir.ActivationFunctionType.Sqrt,
                    bias=eps, scale=1.0,
                )
                nc.vector.reciprocal(out=rs, in_=rs)

                o = work.tile([128, D], f32, tag="o", bufs=8, name=f"o{t}")
                if t >= NT - 2:
                    # tail tiles: normalize on ACT (idle there) to shorten the
                    # serial DVE chain after the last matmul:
                    # o = Identity(x * rstd + (-mean*rstd))
                    negms = stat.tile([128, 1], f32, tag="negms")
                    nc.vector.tensor_scalar(
                        out=negms, in0=mv[:, 0:1], scalar1=rs, scalar2=-1.0,
                        op0=mybir.AluOpType.mult, op1=mybir.AluOpType.mult,
                    )
                    nc.scalar.activation(
                        out=o, in_=x, func=mybir.ActivationFunctionType.Identity,
                        bias=negms, scale=rs,
                    )
                else:
                    nc.vector.tensor_scalar(
                        out=o, in0=x,
                        scalar1=mv[:, 0:1], scalar2=rs,
                        op0=mybir.AluOpType.subtract, op1=mybir.AluOpType.mult,
                    )
                if not trivial_affine:
                    nc.vector.tensor_mul(out=o, in0=o, in1=gam_sb)
                    nc.vector.tensor_add(out=o, in0=o, in1=bet_sb)
                out_tiles.append((t, o))

            # stores last: they queue behind the input stream instead of
            # competing with it for HBM bandwidth mid-kernel
            for t, o in out_tiles:
                nc.scalar.dma_start(out=out_t[t], in_=o)

    _split_excess_waits(nc)
    return nc


def _get_program(add_pos: bool, trivial_affine: bool):
    key = (add_pos, trivial_affine, MM_DTYPE)
    if key not in _PROGS:
        _PROGS[key] = _build_program(add_pos, trivial_affine)
    return _PROGS[key]


def kernel(dec_inp, pos_emb, mems, gamma, beta, add_position):
    from concourse.bass_utils import run_bass_kernel_spmd

    dec_inp = np.asarray(dec_inp, dtype=np.float32)
    pos_emb = np.asarray(pos_emb, dtype=np.float32)
    mems = np.asarray(mems, dtype=np.float32)
    gamma = np.asarray(gamma, dtype=np.float32)
    beta = np.asarray(beta, dtype=np.float32)
    add_pos = bool(int(add_position))
    trivial = bool(np.all(gamma == 1.0) and np.all(beta == 0.0))

    import ml_dtypes

    nc = _get_program(add_pos, trivial)
    wb = _wband().astype(ml_dtypes.bfloat16)
    pos2d = np.ascontiguousarray(pos_emb[:, 0, :])

    in_maps = []
    for b in range(BATCH):
        m = {
            "dec": np.ascontiguousarray(dec_inp[:, b, :]),
            "memsb": np.ascontiguousarray(mems[:, b, :]),
            "wband": wb,
        }
        if add_pos:
            m["pos"] = pos2d
        if not trivial:
            m["gam"] = gamma
            m["bet"] = beta
        in_maps.append(m)

    res = run_bass_kernel_spmd(nc, in_maps, list(range(BATCH)))
    return np.stack([res.results[b]["out"] for b in range(BATCH)], axis=1)


# revision 30
# speedup vs baseline: 1.1031x; 1.0048x over previous
"""Trainium2 Bass kernel for nn_CausalFT (causal Fourier transform + residual + LayerNorm).

reference semantics (QLEN=1024, MLEN=1024, BATCH=8, D_MODEL=1024, klen=2048):
    cat  = concat([mems, dec_inp], axis=0) (+ pos_emb broadcast over batch)
    ft   = einsum('ml,lbd->mbd', ft_matrix(1024, 2048), cat)
    x    = dec_inp + ft / sqrt(2048)
    out  = LayerNorm_d(x) * gamma + beta

Sharding: data-parallel over batch — core b computes out[:, b, :] entirely
(no collectives).  The FT matrix is an input-independent constant, computed
host-side (bit-identical to the reference, via jax on CPU), pre-scaled by
1/sqrt(klen), transposed and band-packed: row-tile t of the output only needs
contraction tiles k in [t, t+8] (the matrix is banded: ft[m, j] != 0 only for
m <= j <= m+1024), which cuts matmul work and weight traffic by ~44%.

Matmuls run in float32r (1 PE cycle/row vs 4 for fp32; measured output
rel-err 7e-6 vs 6e-6 for full fp32 on this problem).
"""

import math

import numpy as np

QLEN, MLEN, BATCH, D = 1024, 1024, 8, 1024
KLEN = QLEN + MLEN
NT = QLEN // 128          # 8 output row tiles
NK = KLEN // 128          # 16 contraction tiles
BW = NK - NT + 1          # 9 band K-tiles per output row tile
LN_EPS = 1e-5

# matmul operand dtype: "f32" (exact, 4 cyc/row) or "f32r" (reduced, 1 cyc/row)
MM_DTYPE = "f32r"

_WBAND = None
_PROGS = {}


def _ft_matrix_np():
    """Replicate reference._ft_matrix bit-for-bit using jax on CPU."""
    import jax
    import jax.numpy as jnp

    cpu = jax.local_devices(backend="cpu")[0]
    with jax.default_device(cpu):
        qlen, klen = QLEN, KLEN
        ft_len = klen - qlen + 1
        m = jnp.arange(qlen, dtype=jnp.float32)
        k = jnp.arange(ft_len, dtype=jnp.float32)
        base = jnp.cos((2.0 * float(np.pi)) * jnp.outer(m, k) / float(ft_len))
        base = base / float(np.sqrt(ft_len))
        mat = jnp.pad(base, ((0, 0), (klen - ft_len, 0)))
        shift = (qlen - 1) - jnp.arange(qlen)
        cols = (jnp.arange(klen)[None, :] + shift[:, None]) % klen
        mat = jnp.take_along_axis(mat, cols, axis=1)
        rows = jnp.arange(qlen)[:, None]
        js = jnp.arange(klen)[None, :]
        mask = (js <= rows + (klen - qlen)) & (js >= rows)
        mat = jnp.where(mask, mat, jnp.float32(0.0))
        return np.asarray(jax.device_get(mat), dtype=np.float32)


def _wband():
    """Band-packed, pre-scaled, transposed FT matrix: [128, NT*BW, 128].

    wband[p, t*BW+i, m] = (W/sqrt(KLEN))[128t+m, 128(t+i)+p]  (lhsT layout).
    """
    global _WBAND
    if _WBAND is None:
        w = _ft_matrix_np() / np.float32(math.sqrt(KLEN))
        wb = np.empty((128, NT * BW, 128), dtype=np.float32)
        for t in range(NT):
            for i in range(BW):
                blk = w[128 * t : 128 * (t + 1), 128 * (t + i) : 128 * (t + i) + 128]
                wb[:, t * BW + i, :] = blk.T
        _WBAND = np.ascontiguousarray(wb)
    return _WBAND


def _install_drain_patch():
    """Work around walrus 'Too many sync wait commands' on the Tile tail drain.

    The stock TileContext._drain_and_barrier emits ONE sync-engine Drain
    carrying a sem wait for every proc lane that ticked (up to 27).  The
    walrus build in this environment accepts only a single sync-wait per
    instruction, so split the global-clock wait set across consecutive
    Drains (one wait each) — sequential execution on the same engine gives
    the same quiescence guarantee.  Also skip the tail per-sem zeroing:
    the bass preamble range-clears every kernel semaphore at program start
    on each execution, so the ~250 walrus-expanded tail EVSEMs (~8us) are
    redundant for re-execution correctness.
    """
    import re

    import bass_rust
    import concourse.tile as _tile
    from concourse.vector_clock import ScopedClock

    if getattr(_tile.TileContext, "_drain_patch_installed", False):
        return

    def _clock_ticks(vc):
        m = re.search(r"\[([0-9, ]*)\]", repr(vc))
        if not m or not m.group(1).strip():
            return []
        return [int(x) for x in m.group(1).split(",")]

    def _patched_drain_and_barrier(self, tick_clock, wait_clock):
        nc = self.nc
        ticks = _clock_ticks(tick_clock.global_clock)
        for i, t in enumerate(ticks):
            if t > 0:
                part = bass_rust.VectorClock()
                part.require_at_least(i, t)
                d = nc.sync.drain()
                wait_clock.add_sem_waits(d.ins, ScopedClock({None: part}))
        assert self.sems is not None
        popped = nc._tile_sem_poison_stack.pop()
        assert popped is self._sem_poison
        nc._state.prepend_free_semaphores(
            [s.num for s in self.sems.allocated().values()]
        )

    _tile.TileContext._drain_and_barrier = _patched_drain_and_barrier
    _tile.TileContext._drain_patch_installed = True


def _split_excess_waits(nc, cap=1):
    """Hoist excess per-instruction sem waits onto preceding same-engine nops.

    The walrus build here accepts only `cap` sync-wait commands per
    instruction.  Engines execute their instruction stream in order, so
    moving waits to immediately-preceding same-engine nops preserves the
    ordering semantics (the instruction still starts only after every wait
    is satisfied).
    """
    import concourse.mybir as mybir

    for bb in nc.main_func.blocks:
        insts = list(bb.instructions)
        if not any(
            i.sync_info and i.sync_info.on_wait and len(i.sync_info.on_wait) > cap
            for i in insts
        ):
            continue
        new = []
        for inst in insts:
            si = inst.sync_info
            waits = list(si.on_wait) if si and si.on_wait else []
            if len(waits) > cap:
                for sw in waits[:-cap]:
                    nop = nc.engines[inst.engine].nop(nofuse=True).ins
                    cur = nc.cur_bb.bb
                    assert cur.instructions and cur.instructions[-1] is nop
                    cur.instructions.pop()
                    nop.sync_info = mybir.SyncInfo(on_wait=[sw], on_update=[])
                    new.append(nop)
                inst.sync_info = mybir.SyncInfo(
                    on_wait=waits[-cap:], on_update=list(si.on_update or [])
                )
            new.append(inst)
        bb.instructions.clear()
        for i in new:
            bb.instructions.append(i)


def _build_program(add_pos: bool, trivial_affine: bool):
    _install_drain_patch()
    import concourse.bass as bass
    import concourse.mybir as mybir
    import concourse.tile as tile

    f32 = mybir.dt.float32
    nc = bass.Bass()

    dec = nc.dram_tensor("dec", [QLEN, D], f32, kind="ExternalInput")
    memsb = nc.dram_tensor("memsb", [MLEN, D], f32, kind="ExternalInput")
    wband = nc.dram_tensor(
        "wband", [128, NT * BW, 128], mybir.dt.float16, kind="ExternalInput"
    )
    pos = None
    if add_pos:
        pos = nc.dram_tensor("pos", [KLEN, D], f32, kind="ExternalInput")
    gam = bet = None
    if not trivial_affine:
        gam = nc.dram_tensor("gam", [D], f32, kind="ExternalInput")
        bet = nc.dram_tensor("bet", [D], f32, kind="ExternalInput")
    out = nc.dram_tensor("out", [QLEN, D], f32, kind="ExternalOutput")

    dec_t = dec.rearrange("(k p) d -> k p d", p=128)      # [8, 128, 1024]
    mems_t = memsb.rearrange("(k p) d -> k p d", p=128)   # [8, 128, 1024]
    out_t = out.rearrange("(t p) d -> t p d", p=128)      # [8, 128, 1024]

    # FP32r: the PE runs 4x faster, but the BIR verifier requires every
    # producer feeding an fp32r matmul to emit fp32r — so type the weight and
    # rhs tiles (and the DMA source APs, a pure bitcast) as float32r.
    mm_dt = f32 if MM_DTYPE == "f32" else mybir.dt.float32r
    src_cast = (lambda ap: ap) if MM_DTYPE == "f32" else (
        lambda ap: ap.bitcast(mybir.dt.float32r)
    )

    with tile.TileContext(nc) as tc:
        with (
            tc.tile_pool(name="big", bufs=1) as big,
            tc.tile_pool(name="posp", bufs=2) as posp,
            tc.tile_pool(name="small", bufs=1) as small,
            tc.tile_pool(name="work", bufs=3) as work,
            tc.tile_pool(name="stat", bufs=4) as stat,
            tc.tile_pool(name="ps", bufs=2, space="PSUM") as ps,
        ):
            eps = small.tile([128, 1], f32)
            nc.vector.memset(eps, LN_EPS)
            gam_sb = bet_sb = None
            if not trivial_affine:
                gam_sb = small.tile([128, D], f32)
                bet_sb = small.tile([128, D], f32)
                gam_ap, bet_ap = gam[:], bet[:]
                nc.sync.dma_start(
                    out=gam_sb,
                    in_=bass.AP(tensor=gam_ap.tensor, offset=0, ap=[[0, 128]] + list(gam_ap.ap)),
                )
                nc.sync.dma_start(
                    out=bet_sb,
                    in_=bass.AP(tensor=bet_ap.tensor, offset=0, ap=[[0, 128]] + list(bet_ap.ap)),
                )

            # --- weights: band-packed lhsT, shipped as fp16 (half the HBM
            # bytes; fp16's 10-bit mantissa fits W's tiny uniform scale, so the
            # rounding adds only ~1e-5 rel err on the output) and up-converted to f32r on GpSimd.
            # Chunk t == the 9 tiles of matmul group t, so conversion order
            # matches consumption order.  t=0 first, rest after mems/pos. ---
            wb = big.tile([128, NT * BW, 128], mm_dt)

            def wb_chunk(t):
                wst = posp.tile([128, BW, 128], mybir.dt.float16, tag="wst", bufs=2)
                nc.gpsimd.dma_start(out=wst, in_=wband[:, BW * t : BW * (t + 1), :])
                nc.scalar.copy(out=wb[:, BW * t : BW * (t + 1), :], in_=wst)

            wb_chunk(0)

            # --- rhs: catp[k] = cat tile k (+ pos), decraw for the residual.
            # Interleave mems/pos loads and the pos-adds per k so low-k catp
            # tiles (which gate the first matmul groups) complete earliest.
            # catp as a sliding window: tile k is dead once group k's matmuls
            # finish, so 12 rotating slots suffice (9-tile band + margin);
            # the freed SBUF holds all 8 output tiles so stores can be
            # emitted last and stop stealing HBM bandwidth from the input
            # stream mid-kernel.
            catp_t = []
            decraw = big.tile([128, NT, D], f32)
            pos_k = pos.rearrange("(k p) d -> k p d", p=128) if add_pos else None

            def new_ck(k):
                ck = work.tile([128, D], mm_dt, tag="ck", bufs=12, name=f"ck{k}")
                catp_t.append(ck)
                return ck

            def pos_add(k, ck, in0):
                # GpSimd takes every third add (2x slower than DVE but runs
                # in parallel; the add chain paces the matmul groups)
                pc = posp.tile([128, D], f32, tag="pos", bufs=4)
                nc.scalar.dma_start(out=pc, in_=pos_k[k])
                eng = nc.gpsimd if k % 3 == 2 else nc.vector
                eng.tensor_add(out=ck, in0=in0, in1=pc)

            for k in range(8):
                ck = new_ck(k)
                nc.sync.dma_start(out=ck, in_=src_cast(mems_t[k]))
                if add_pos:
                    pos_add(k, ck, ck)
            ck = new_ck(8)
            nc.sync.dma_start(out=decraw[:, 0, :], in_=dec_t[0])
            if add_pos:
                pos_add(8, ck, decraw[:, 0, :])
            else:
                nc.gpsimd.tensor_copy(out=ck, in_=decraw[:, 0, :])
            for t in (1, 2, 3, 4):
                wb_chunk(t)
            for k in range(9, 16):
                ck = new_ck(k)
                nc.sync.dma_start(out=decraw[:, k - 8, :], in_=dec_t[k - 8])
                if add_pos:
                    pos_add(k, ck, decraw[:, k - 8, :])
                else:
                    nc.gpsimd.tensor_copy(out=ck, in_=decraw[:, k - 8, :])
            for t in (5, 6, 7):
                wb_chunk(t)

            # --- band matmul + fused residual/LayerNorm epilogue per row tile ---
            out_tiles = []
            for t in range(NT):
                psA = ps.tile([128, 512], f32, tag="A", bufs=4)
                psB = ps.tile([128, 512], f32, tag="B", bufs=4)
                for i in range(BW):
                    k = t + i
                    wt = wb[:, t * BW + i, :]
                    nc.tensor.matmul(
                        psA, wt, catp_t[k][:, 0:512], start=(i == 0), stop=(i == BW - 1)
                    )
                    nc.tensor.matmul(
                        psB, wt, catp_t[k][:, 512:1024], start=(i == 0), stop=(i == BW - 1)
                    )

                x = work.tile([128, D], f32, tag="x", bufs=4)
                nc.vector.tensor_add(out=x[:, 0:512], in0=psA, in1=decraw[:, t, 0:512])
                nc.vector.tensor_add(
                    out=x[:, 512:1024], in0=psB, in1=decraw[:, t, 512:1024]
                )

                st = stat.tile([128, 2, 6], f32, tag="st")
                nc.vector.bn_stats(out=st[:, 0, :], in_=x[:, 0:512])
                nc.vector.bn_stats(out=st[:, 1, :], in_=x[:, 512:1024])
                mv = stat.tile([128, 2], f32, tag="mv")
                nc.vector.bn_aggr(out=mv, in_=st)
                rs = stat.tile([128, 1], f32, tag="rs")
                nc.scalar.activation(
                    out=rs, in_=mv[:, 1:2],
                    func=mybir.ActivationFunctionType.Sqrt,
                    bias=eps, scale=1.0,
                )
                nc.vector.reciprocal(out=rs, in_=rs)

                o = work.tile([128, D], f32, tag="o", bufs=8, name=f"o{t}")
                if t >= NT - 2:
                    # tail tiles: normalize on ACT (idle there) to shorten the
                    # serial DVE chain after the last matmul:
                    # o = Identity(x * rstd + (-mean*rstd))
                    negms = stat.tile([128, 1], f32, tag="negms")
                    nc.vector.tensor_scalar(
                        out=negms, in0=mv[:, 0:1], scalar1=rs, scalar2=-1.0,
                        op0=mybir.AluOpType.mult, op1=mybir.AluOpType.mult,
                    )
                    nc.scalar.activation(
                        out=o, in_=x, func=mybir.ActivationFunctionType.Identity,
                        bias=negms, scale=rs,
                    )
                else:
                    nc.vector.tensor_scalar(
                        out=o, in0=x,
                        scalar1=mv[:, 0:1], scalar2=rs,
                        op0=mybir.AluOpType.subtract, op1=mybir.AluOpType.mult,
                    )
                if not trivial_affine:
                    nc.vector.tensor_mul(out=o, in0=o, in1=gam_sb)
                    nc.vector.tensor_add(out=o, in0=o, in1=bet_sb)
                out_tiles.append((t, o))

            # stores last: they queue behind the input stream instead of
            # competing with it for HBM bandwidth mid-kernel
            for t, o in out_tiles:
                nc.scalar.dma_start(out=out_t[t], in_=o)

    _split_excess_waits(nc)
    return nc


def _get_program(add_pos: bool, trivial_affine: bool):
    key = (add_pos, trivial_affine, MM_DTYPE)
    if key not in _PROGS:
        _PROGS[key] = _build_program(add_pos, trivial_affine)
    return _PROGS[key]


def kernel(dec_inp, pos_emb, mems, gamma, beta, add_position):
    from concourse.bass_utils import run_bass_kernel_spmd

    dec_inp = np.asarray(dec_inp, dtype=np.float32)
    pos_emb = np.asarray(pos_emb, dtype=np.float32)
    mems = np.asarray(mems, dtype=np.float32)
    gamma = np.asarray(gamma, dtype=np.float32)
    beta = np.asarray(beta, dtype=np.float32)
    add_pos = bool(int(add_position))
    trivial = bool(np.all(gamma == 1.0) and np.all(beta == 0.0))

    nc = _get_program(add_pos, trivial)
    wb = _wband().astype(np.float16)
    pos2d = np.ascontiguousarray(pos_emb[:, 0, :])

    in_maps = []
    for b in range(BATCH):
        m = {
            "dec": np.ascontiguousarray(dec_inp[:, b, :]),
            "memsb": np.ascontiguousarray(mems[:, b, :]),
            "wband": wb,
        }
        if add_pos:
            m["pos"] = pos2d
        if not trivial:
            m["gam"] = gamma
            m["bet"] = beta
        in_maps.append(m)

    res = run_bass_kernel_spmd(nc, in_maps, list(range(BATCH)))
    return np.stack([res.results[b]["out"] for b in range(BATCH)], axis=1)


# revision 31
# speedup vs baseline: 1.1938x; 1.0823x over previous
"""Trainium2 Bass kernel for nn_CausalFT (causal Fourier transform + residual + LayerNorm).

reference semantics (QLEN=1024, MLEN=1024, BATCH=8, D_MODEL=1024, klen=2048):
    cat  = concat([mems, dec_inp], axis=0) (+ pos_emb broadcast over batch)
    ft   = einsum('ml,lbd->mbd', ft_matrix(1024, 2048), cat)
    x    = dec_inp + ft / sqrt(2048)
    out  = LayerNorm_d(x) * gamma + beta

Sharding: data-parallel over batch — core b computes out[:, b, :] entirely
(no collectives).  The FT matrix is an input-independent constant, computed
host-side (bit-identical to the reference, via jax on CPU), pre-scaled by
1/sqrt(klen), transposed and band-packed: row-tile t of the output only needs
contraction tiles k in [t, t+8] (the matrix is banded: ft[m, j] != 0 only for
m <= j <= m+1024), which cuts matmul work and weight traffic by ~44%.

Matmuls run in float32r (1 PE cycle/row vs 4 for fp32; measured output
rel-err 7e-6 vs 6e-6 for full fp32 on this problem).
"""

import math

import numpy as np

QLEN, MLEN, BATCH, D = 1024, 1024, 8, 1024
KLEN = QLEN + MLEN
NT = QLEN // 128          # 8 output row tiles
NK = KLEN // 128          # 16 contraction tiles
BW = NK - NT + 1          # 9 band K-tiles per output row tile
LN_EPS = 1e-5

# matmul operand dtype: "f32" (exact, 4 cyc/row) or "f32r" (reduced, 1 cyc/row)
MM_DTYPE = "f32r"

_WBAND = None
_PROGS = {}


def _ft_matrix_np():
    """Replicate reference._ft_matrix bit-for-bit using jax on CPU."""
    import jax
    import jax.numpy as jnp

    cpu = jax.local_devices(backend="cpu")[0]
    with jax.default_device(cpu):
        qlen, klen = QLEN, KLEN
        ft_len = klen - qlen + 1
        m = jnp.arange(qlen, dtype=jnp.float32)
        k = jnp.arange(ft_len, dtype=jnp.float32)
        base = jnp.cos((2.0 * float(np.pi)) * jnp.outer(m, k) / float(ft_len))
        base = base / float(np.sqrt(ft_len))
        mat = jnp.pad(base, ((0, 0), (klen - ft_len, 0)))
        shift = (qlen - 1) - jnp.arange(qlen)
        cols = (jnp.arange(klen)[None, :] + shift[:, None]) % klen
        mat = jnp.take_along_axis(mat, cols, axis=1)
        rows = jnp.arange(qlen)[:, None]
        js = jnp.arange(klen)[None, :]
        mask = (js <= rows + (klen - qlen)) & (js >= rows)
        mat = jnp.where(mask, mat, jnp.float32(0.0))
        return np.asarray(jax.device_get(mat), dtype=np.float32)


def _wband():
    """Band-packed, pre-scaled, transposed FT matrix: [128, NT*BW, 128].

    wband[p, t*BW+i, m] = (W/sqrt(KLEN))[128t+m, 128(t+i)+p]  (lhsT layout).
    """
    global _WBAND
    if _WBAND is None:
        w = _ft_matrix_np() / np.float32(math.sqrt(KLEN))
        wb = np.empty((128, NT * BW, 128), dtype=np.float32)
        for t in range(NT):
            for i in range(BW):
                blk = w[128 * t : 128 * (t + 1), 128 * (t + i) : 128 * (t + i) + 128]
                wb[:, t * BW + i, :] = blk.T
        _WBAND = np.ascontiguousarray(wb)
    return _WBAND


def _install_drain_patch():
    """Work around walrus 'Too many sync wait commands' on the Tile tail drain.

    The stock TileContext._drain_and_barrier emits ONE sync-engine Drain
    carrying a sem wait for every proc lane that ticked (up to 27).  The
    walrus build in this environment accepts only a single sync-wait per
    instruction, so split the global-clock wait set across consecutive
    Drains (one wait each) — sequential execution on the same engine gives
    the same quiescence guarantee.  Also skip the tail per-sem zeroing:
    the bass preamble range-clears every kernel semaphore at program start
    on each execution, so the ~250 walrus-expanded tail EVSEMs (~8us) are
    redundant for re-execution correctness.
    """
    import re

    import bass_rust
    import concourse.tile as _tile
    from concourse.vector_clock import ScopedClock

    if getattr(_tile.TileContext, "_drain_patch_installed", False):
        return

    def _clock_ticks(vc):
        m = re.search(r"\[([0-9, ]*)\]", repr(vc))
        if not m or not m.group(1).strip():
            return []
        return [int(x) for x in m.group(1).split(",")]

    def _patched_drain_and_barrier(self, tick_clock, wait_clock):
        nc = self.nc
        ticks = _clock_ticks(tick_clock.global_clock)
        for i, t in enumerate(ticks):
            if t > 0:
                part = bass_rust.VectorClock()
                part.require_at_least(i, t)
                d = nc.sync.drain()
                wait_clock.add_sem_waits(d.ins, ScopedClock({None: part}))
        assert self.sems is not None
        popped = nc._tile_sem_poison_stack.pop()
        assert popped is self._sem_poison
        nc._state.prepend_free_semaphores(
            [s.num for s in self.sems.allocated().values()]
        )

    _tile.TileContext._drain_and_barrier = _patched_drain_and_barrier
    _tile.TileContext._drain_patch_installed = True


def _split_excess_waits(nc, cap=1):
    """Hoist excess per-instruction sem waits onto preceding same-engine nops.

    The walrus build here accepts only `cap` sync-wait commands per
    instruction.  Engines execute their instruction stream in order, so
    moving waits to immediately-preceding same-engine nops preserves the
    ordering semantics (the instruction still starts only after every wait
    is satisfied).
    """
    import concourse.mybir as mybir

    for bb in nc.main_func.blocks:
        insts = list(bb.instructions)
        if not any(
            i.sync_info and i.sync_info.on_wait and len(i.sync_info.on_wait) > cap
            for i in insts
        ):
            continue
        new = []
        for inst in insts:
            si = inst.sync_info
            waits = list(si.on_wait) if si and si.on_wait else []
            if len(waits) > cap:
                for sw in waits[:-cap]:
                    nop = nc.engines[inst.engine].nop(nofuse=True).ins
                    cur = nc.cur_bb.bb
                    assert cur.instructions and cur.instructions[-1] is nop
                    cur.instructions.pop()
                    nop.sync_info = mybir.SyncInfo(on_wait=[sw], on_update=[])
                    new.append(nop)
                inst.sync_info = mybir.SyncInfo(
                    on_wait=waits[-cap:], on_update=list(si.on_update or [])
                )
            new.append(inst)
        bb.instructions.clear()
        for i in new:
            bb.instructions.append(i)


def _build_program(add_pos: bool, trivial_affine: bool):
    _install_drain_patch()
    import concourse.bass as bass
    import concourse.mybir as mybir
    import concourse.tile as tile

    f32 = mybir.dt.float32
    nc = bass.Bass()

    f16 = mybir.dt.float16
    dec = nc.dram_tensor("dec", [QLEN, D], f32, kind="ExternalInput")
    # mems and pos only feed the FT channel, which enters the output scaled
    # by 1/sqrt(klen) (~0.02 of x's unit scale) — fp16 shipping adds <1e-5
    # rel err while halving their HBM bytes
    memsb = nc.dram_tensor("memsb", [MLEN, D], f16, kind="ExternalInput")
    wband = nc.dram_tensor(
        "wband", [128, NT * BW, 128], mybir.dt.float16, kind="ExternalInput"
    )
    pos = None
    if add_pos:
        pos = nc.dram_tensor("pos", [KLEN, D], f16, kind="ExternalInput")
    gam = bet = None
    if not trivial_affine:
        gam = nc.dram_tensor("gam", [D], f32, kind="ExternalInput")
        bet = nc.dram_tensor("bet", [D], f32, kind="ExternalInput")
    out = nc.dram_tensor("out", [QLEN, D], f32, kind="ExternalOutput")

    dec_t = dec.rearrange("(k p) d -> k p d", p=128)      # [8, 128, 1024]
    mems_t = memsb.rearrange("(k p) d -> k p d", p=128)   # [8, 128, 1024]
    out_t = out.rearrange("(t p) d -> t p d", p=128)      # [8, 128, 1024]

    # FP32r: the PE runs 4x faster, but the BIR verifier requires every
    # producer feeding an fp32r matmul to emit fp32r — so type the weight and
    # rhs tiles (and the DMA source APs, a pure bitcast) as float32r.
    mm_dt = f32 if MM_DTYPE == "f32" else mybir.dt.float32r
    src_cast = (lambda ap: ap) if MM_DTYPE == "f32" else (
        lambda ap: ap.bitcast(mybir.dt.float32r)
    )

    with tile.TileContext(nc) as tc:
        with (
            tc.tile_pool(name="big", bufs=1) as big,
            tc.tile_pool(name="posp", bufs=2) as posp,
            tc.tile_pool(name="small", bufs=1) as small,
            tc.tile_pool(name="work", bufs=3) as work,
            tc.tile_pool(name="stat", bufs=4) as stat,
            tc.tile_pool(name="ps", bufs=2, space="PSUM") as ps,
        ):
            eps = small.tile([128, 1], f32)
            nc.vector.memset(eps, LN_EPS)
            gam_sb = bet_sb = None
            if not trivial_affine:
                gam_sb = small.tile([128, D], f32)
                bet_sb = small.tile([128, D], f32)
                gam_ap, bet_ap = gam[:], bet[:]
                nc.sync.dma_start(
                    out=gam_sb,
                    in_=bass.AP(tensor=gam_ap.tensor, offset=0, ap=[[0, 128]] + list(gam_ap.ap)),
                )
                nc.sync.dma_start(
                    out=bet_sb,
                    in_=bass.AP(tensor=bet_ap.tensor, offset=0, ap=[[0, 128]] + list(bet_ap.ap)),
                )

            # --- weights: band-packed lhsT, shipped as fp16 (half the HBM
            # bytes; fp16's 10-bit mantissa fits W's tiny uniform scale, so the
            # rounding adds only ~1e-5 rel err on the output) and up-converted to f32r on GpSimd.
            # Chunk t == the 9 tiles of matmul group t, so conversion order
            # matches consumption order.  t=0 first, rest after mems/pos. ---
            wb = big.tile([128, NT * BW, 128], mm_dt)

            def wb_chunk(t):
                wst = posp.tile([128, BW, 128], mybir.dt.float16, tag="wst", bufs=2)
                nc.gpsimd.dma_start(out=wst, in_=wband[:, BW * t : BW * (t + 1), :])
                nc.scalar.copy(out=wb[:, BW * t : BW * (t + 1), :], in_=wst)

            wb_chunk(0)

            # --- rhs: catp[k] = cat tile k (+ pos), decraw for the residual.
            # Interleave mems/pos loads and the pos-adds per k so low-k catp
            # tiles (which gate the first matmul groups) complete earliest.
            # catp as a sliding window: tile k is dead once group k's matmuls
            # finish, so 12 rotating slots suffice (9-tile band + margin);
            # the freed SBUF holds all 8 output tiles so stores can be
            # emitted last and stop stealing HBM bandwidth from the input
            # stream mid-kernel.
            catp_t = []
            decraw = big.tile([128, NT, D], f32)
            pos_k = pos.rearrange("(k p) d -> k p d", p=128) if add_pos else None

            def new_ck(k):
                ck = work.tile([128, D], mm_dt, tag="ck", bufs=12, name=f"ck{k}")
                catp_t.append(ck)
                return ck

            def pos_add(k, ck, in0):
                # GpSimd takes every third add (2x slower than DVE but runs
                # in parallel; the add chain paces the matmul groups)
                pc = posp.tile([128, D], f16, tag="pos", bufs=4)
                nc.scalar.dma_start(out=pc, in_=pos_k[k])
                eng = nc.gpsimd if k % 3 == 2 else nc.vector
                eng.tensor_add(out=ck, in0=in0, in1=pc)

            for k in range(8):
                ck = new_ck(k)
                ms = posp.tile([128, D], f16, tag="ms", bufs=3)
                nc.sync.dma_start(out=ms, in_=mems_t[k])
                if add_pos:
                    pos_add(k, ck, ms)
                else:
                    nc.scalar.copy(out=ck, in_=ms)
            ck = new_ck(8)
            nc.sync.dma_start(out=decraw[:, 0, :], in_=dec_t[0])
            if add_pos:
                pos_add(8, ck, decraw[:, 0, :])
            else:
                nc.gpsimd.tensor_copy(out=ck, in_=decraw[:, 0, :])
            for t in (1, 2, 3, 4):
                wb_chunk(t)
            for k in range(9, 16):
                ck = new_ck(k)
                nc.sync.dma_start(out=decraw[:, k - 8, :], in_=dec_t[k - 8])
                if add_pos:
                    pos_add(k, ck, decraw[:, k - 8, :])
                else:
                    nc.gpsimd.tensor_copy(out=ck, in_=decraw[:, k - 8, :])
            for t in (5, 6, 7):
                wb_chunk(t)

            # --- band matmul + fused residual/LayerNorm epilogue per row tile ---
            out_tiles = []
            for t in range(NT):
                psA = ps.tile([128, 512], f32, tag="A", bufs=4)
                psB = ps.tile([128, 512], f32, tag="B", bufs=4)
                for i in range(BW):
                    k = t + i
                    wt = wb[:, t * BW + i, :]
                    nc.tensor.matmul(
                        psA, wt, catp_t[k][:, 0:512], start=(i == 0), stop=(i == BW - 1)
                    )
                    nc.tensor.matmul(
                        psB, wt, catp_t[k][:, 512:1024], start=(i == 0), stop=(i == BW - 1)
                    )

                x = work.tile([128, D], f32, tag="x", bufs=4)
                nc.vector.tensor_add(out=x[:, 0:512], in0=psA, in1=decraw[:, t, 0:512])
                nc.vector.tensor_add(
                    out=x[:, 512:1024], in0=psB, in1=decraw[:, t, 512:1024]
                )

                st = stat.tile([128, 2, 6], f32, tag="st")
                nc.vector.bn_stats(out=st[:, 0, :], in_=x[:, 0:512])
                nc.vector.bn_stats(out=st[:, 1, :], in_=x[:, 512:1024])
                mv = stat.tile([128, 2], f32, tag="mv")
                nc.vector.bn_aggr(out=mv, in_=st)
                rs = stat.tile([128, 1], f32, tag="rs")
                nc.scalar.activation(
                    out=rs, in_=mv[:, 1:2],
                    func=mybir.ActivationFunctionType.Sqrt,
                    bias=eps, scale=1.0,
                )
                nc.vector.reciprocal(out=rs, in_=rs)

                o = work.tile([128, D], f32, tag="o", bufs=8, name=f"o{t}")
                if t >= NT - 2:
                    # tail tiles: normalize on ACT (idle there) to shorten the
                    # serial DVE chain after the last matmul:
                    # o = Identity(x * rstd + (-mean*rstd))
                    negms = stat.tile([128, 1], f32, tag="negms")
                    nc.vector.tensor_scalar(
                        out=negms, in0=mv[:, 0:1], scalar1=rs, scalar2=-1.0,
                        op0=mybir.AluOpType.mult, op1=mybir.AluOpType.mult,
                    )
                    nc.scalar.activation(
                        out=o, in_=x, func=mybir.ActivationFunctionType.Identity,
                        bias=negms, scale=rs,
                    )
                else:
                    nc.vector.tensor_scalar(
                        out=o, in0=x,
                        scalar1=mv[:, 0:1], scalar2=rs,
                        op0=mybir.AluOpType.subtract, op1=mybir.AluOpType.mult,
                    )
                if not trivial_affine:
                    nc.vector.tensor_mul(out=o, in0=o, in1=gam_sb)
                    nc.vector.tensor_add(out=o, in0=o, in1=bet_sb)
                out_tiles.append((t, o))

            # stores last: they queue behind the input stream instead of
            # competing with it for HBM bandwidth mid-kernel
            for t, o in out_tiles:
                nc.scalar.dma_start(out=out_t[t], in_=o)

    _split_excess_waits(nc)
    return nc


def _get_program(add_pos: bool, trivial_affine: bool):
    key = (add_pos, trivial_affine, MM_DTYPE)
    if key not in _PROGS:
        _PROGS[key] = _build_program(add_pos, trivial_affine)
    return _PROGS[key]


def kernel(dec_inp, pos_emb, mems, gamma, beta, add_position):
    from concourse.bass_utils import run_bass_kernel_spmd

    dec_inp = np.asarray(dec_inp, dtype=np.float32)
    pos_emb = np.asarray(pos_emb, dtype=np.float32)
    mems = np.asarray(mems, dtype=np.float32)
    gamma = np.asarray(gamma, dtype=np.float32)
    beta = np.asarray(beta, dtype=np.float32)
    add_pos = bool(int(add_position))
    trivial = bool(np.all(gamma == 1.0) and np.all(beta == 0.0))

    nc = _get_program(add_pos, trivial)
    wb = _wband().astype(np.float16)
    pos2d = np.ascontiguousarray(pos_emb[:, 0, :]).astype(np.float16)

    in_maps = []
    for b in range(BATCH):
        m = {
            "dec": np.ascontiguousarray(dec_inp[:, b, :]),
            "memsb": np.ascontiguousarray(mems[:, b, :]).astype(np.float16),
            "wband": wb,
        }
        if add_pos:
            m["pos"] = pos2d
        if not trivial:
            m["gam"] = gamma
            m["bet"] = beta
        in_maps.append(m)

    res = run_bass_kernel_spmd(nc, in_maps, list(range(BATCH)))
    return np.stack([res.results[b]["out"] for b in range(BATCH)], axis=1)
